# revision 15
# baseline (speedup 1.0000x reference)
"""Trainium2 Bass kernel for nn_CustomizedLinear (masked pathway linear).

out[b, p*768+e] = sum_d x[b,d] * (weight*mask.T)[p,d] * G[d,e] + bias[p]
with B=64, P=256, D=2000, E=768.

Sharding: tensor-parallel over the pathway dim P — 32 pathways per core on
8 cores, paired into 16 pathway-pairs (M = 2*64 = 128 PE rows each).

Chain layout (the key DMA optimization): the 16 pairs are ordered in a
max-overlap chain; gene rows are laid out so that pair i's contraction
window is EXACTLY 6 k-tiles (768 rows) starting at tile 4i.  Consecutive
pairs overlap on a 2-tile (256-row) junction block that holds their shared
genes once.  Chain advance is 4 tiles/pair -> 66 tiles total instead of
~96+partials with per-pair gathers: ~25% less G DMA, no ragged partial
transfers, no Pool memsets, and the PE stays at its floor of exactly
16 pairs x 6 k-tiles x 2 chunks of N=384.

Genes shared with non-adjacent pairs are duplicated; each (pair, gene) has
one designated slot inside the pair's span and the pair's w vector is
nonzero only there (foreign/duplicate/pad rows get w=0, so their G content
cannot leak in).

Dtypes: strips (x*w, stationary) bf16 on DVE; G (moving) fp8 e3m4 scaled
into normal range with the inverse folded into the bf16 weights; PSUM f32;
outputs evicted to bf16 by Act; bias added on host.  PE warmup matmuls
keep the p-state ramp hot during the DMA lead-in.
"""
import sys

sys.path.insert(0, "/opt/trn_rl_repo")

import numpy as np
import ml_dtypes
from contextlib import ExitStack

import concourse.bacc as bacc
import concourse.tile as tile
import concourse.mybir as mybir
from concourse.bass_utils import run_bass_kernel_spmd

F32 = mybir.dt.float32
BF16 = mybir.dt.bfloat16
F8E3 = mybir.dt.float8e3
I16 = mybir.dt.int16

NP_BF16 = ml_dtypes.bfloat16
NP_F8E3 = ml_dtypes.float8_e3m4

N_CORES = 8
B = 64          # batch
D = 2000        # genes (contraction)
E = 768         # embedding
P_TOT = 256     # pathways
P_CORE = P_TOT // N_CORES        # 32 pathways per core
NPAIR = P_CORE // 2              # 16 pathway pairs per core
T_SPAN = 6                       # k-tiles per pair span (fits u<=768)
ADV = 4                          # chain tile advance per pair
NTILE = ADV * (NPAIR - 1) + T_SPAN   # 66 chain tiles
JCAP = 2 * 128                   # junction capacity (2 shared tiles)
NCH = 2                          # PSUM chunks per pair
NC_W = (512, 256)                # chunk widths (asymmetric: short tail)
NC_OFF = (0, 512, 768)
OUT_GROUPS = [(0, 4), (4, 4), (8, 4), (12, 2), (14, 1), (15, 1)]
N_WARM = 5                       # PE warmup matmuls (512 rows each)
# input stream chunking (tile ranges), interleaved x/g on one queue so G
# tile t arrives just ahead of the PE and x stays ahead of the DVE strips
W_BYTES = NPAIR * T_SPAN * 2 * 2         # bf16 w block at head of xw param
X_CHUNKS = [(6, 18), (18, 66)]           # tiles 0-6 ride with w in the head DMA
G_CHUNKS = [(0, 3), (3, 6), (6, 10), (10, 14), (14, 18), (18, 22),
            (22, 28), (28, 34), (34, 42), (42, 50), (50, 58), (58, 66)]
# queue order: w, x0, g0, g1, x1, g2, g3, x2, g4, g5, ... (rest of g)


def _group_of(j):
    for q, (g0, gsz) in enumerate(OUT_GROUPS):
        if g0 <= j < g0 + gsz:
            return q, g0, gsz
    raise ValueError(j)


def _build_program():
    nc = bacc.Bacc()
    g_d = nc.declare_dram_parameter("g", [128, NTILE * E], F8E3,
                                    isOutput=False)
    xw_d = nc.declare_dram_parameter("xw", [128, W_BYTES + NTILE * B], F8E3,
                                     isOutput=False)
    out_d = nc.declare_dram_parameter("out", [NPAIR * 2 * B, E], BF16,
                                      isOutput=True)
    # dst dims [i(2), b(64), slot, e] to match SBUF src [part=(i,b), slot, e]
    out_v = out_d[:].rearrange("(s i b) e -> i b s e", i=2, b=B)

    with tile.TileContext(nc) as tc, ExitStack() as ctx:
        gp = ctx.enter_context(tc.tile_pool(name="gp", bufs=1))
        xp = ctx.enter_context(tc.tile_pool(name="xp", bufs=1))
        stp = ctx.enter_context(tc.tile_pool(name="stp", bufs=4))
        op = ctx.enter_context(tc.tile_pool(name="op", bufs=1))
        psum = ctx.enter_context(tc.tile_pool(name="psum", bufs=3,
                                              space="PSUM"))
        psw = ctx.enter_context(tc.tile_pool(name="psw", bufs=1,
                                             space="PSUM"))

        gbig = gp.tile([128, NTILE * E], F8E3, name="gbig")
        xwbig = xp.tile([128, W_BYTES + NTILE * B], F8E3, name="xwbig")
        wview = xwbig[:, :W_BYTES].bitcast(BF16)   # [128, NPAIR*T_SPAN*2]

        # warm the PE p-state during the DMA lead-in: dummy matmuls on a
        # zeroed tile bridge into the real stream so it starts fully ramped
        wz = xp.tile([128, 512], BF16, name="warmz")
        nc.gpsimd.memset(wz[:], 0)
        pw = psw.tile([128, 512], F32, tag="warm", name="warm")
        for w in range(N_WARM):
            nc.tensor.matmul(pw[:], wz[:, :128], wz[:], start=True,
                             stop=True)

        def dma_x(t0, t1):
            nc.sync.dma_start(
                out=xwbig[:, W_BYTES + t0 * B:W_BYTES + t1 * B],
                in_=xw_d[:, W_BYTES + t0 * B:W_BYTES + t1 * B])

        def dma_g(t0, t1):
            nc.sync.dma_start(out=gbig[:, t0 * E:t1 * E],
                              in_=g_d[:, t0 * E:t1 * E])

        # interleaved input stream: the 768B head DMA (w + x tiles 0-6) and
        # fine G chunks let pair 0 start early; x slabs stay ahead of DVE
        nc.sync.dma_start(out=xwbig[:, :W_BYTES + 6 * B],
                          in_=xw_d[:, :W_BYTES + 6 * B])
        dma_g(*G_CHUNKS[0])
        dma_g(*G_CHUNKS[1])
        dma_x(*X_CHUNKS[0])
        dma_g(*G_CHUNKS[2])
        dma_g(*G_CHUNKS[3])
        dma_x(*X_CHUNKS[1])
        for ch in G_CHUNKS[4:]:
            dma_g(*ch)

        o_tiles = {}

        def compute_pair(j):
            base = j * ADV
            st = stp.tile([128, T_SPAN * 128], BF16, tag="st", name=f"st{j}")
            st3 = st[:].rearrange("p (t i b) -> p t i b", t=T_SPAN, i=2)
            xg_v = (xwbig[:, W_BYTES + base * B:W_BYTES + (base + T_SPAN) * B]
                    .rearrange("p (t b) -> p t b", t=T_SPAN)
                    .unsqueeze(2).broadcast_to([128, T_SPAN, 2, B]))
            wg_v = (wview[:, j * T_SPAN * 2:(j + 1) * T_SPAN * 2]
                    .rearrange("p (t i) -> p t i", t=T_SPAN)
                    .unsqueeze(3).broadcast_to([128, T_SPAN, 2, B]))
            nc.vector.tensor_mul(st3, xg_v, wg_v)

            ps = [psum.tile([128, NC_W[n]], F32, tag=f"ps{n}",
                            name=f"ps{j}_{n}") for n in range(NCH)]

            def rhs(t, n):
                o = (base + t) * E + NC_OFF[n]
                return gbig[:, o:o + NC_W[n]]

            q, g0, gsz = _group_of(j)
            r = j - g0
            if r == 0:
                o_tiles[q] = op.tile([128, gsz * E], BF16, tag=f"o{q}",
                                     name=f"o{q}")
            o_tile = o_tiles[q]
            last_pair = j == NPAIR - 1
            if last_pair:
                # chunk-outer loop: chunk 0 (512 wide) finishes 6 matmuls
                # early so its eviction+DMA overlap chunk 1's matmuls; the
                # final 256-wide chunk keeps the tail eviction+transfer short
                for n in range(NCH):
                    c0, c1 = NC_OFF[n], NC_OFF[n + 1]
                    for t in range(T_SPAN):
                        nc.tensor.matmul(ps[n][:], st[:, 128 * t:128 * (t + 1)],
                                         rhs(t, n), start=(t == 0),
                                         stop=(t == T_SPAN - 1))
                    nc.scalar.activation(
                        o_tile[:, r * E + c0:r * E + c1],
                        ps[n][:], mybir.ActivationFunctionType.Identity)
                    nc.sync.dma_start(
                        out=out_v[:, :, j:j + 1, c0:c1],
                        in_=o_tile[:, r * E + c0:r * E + c1])
                return
            for t in range(T_SPAN):
                for n in range(NCH):
                    nc.tensor.matmul(ps[n][:], st[:, 128 * t:128 * (t + 1)],
                                     rhs(t, n), start=(t == 0),
                                     stop=(t == T_SPAN - 1))
            for n in range(NCH):
                nc.scalar.activation(
                    o_tile[:, r * E + NC_OFF[n]:r * E + NC_OFF[n + 1]],
                    ps[n][:], mybir.ActivationFunctionType.Identity)
            if r == gsz - 1:
                src = o_tile[:].rearrange("p (s e) -> p s e", s=gsz)
                nc.sync.dma_start(out=out_v[:, :, g0:g0 + gsz, :], in_=src)

        for j in range(NPAIR):
            compute_pair(j)
    nc.finalize()
    return nc


_NC_CACHE = None


def _get_program():
    global _NC_CACHE
    if _NC_CACHE is None:
        _NC_CACHE = _build_program()
    return _NC_CACHE


def _pair_pathways(maskc):
    """Pair the 32 local pathways to minimize summed union sizes: greedy
    max-overlap seed, then 2-opt member swaps."""
    m = maskc.astype(np.int32)
    ov = m.T @ m                      # [32, 32] overlap counts
    sz = np.diag(ov).copy()
    cand = [(-ov[a, b], a, b) for a in range(P_CORE)
            for b in range(a + 1, P_CORE)]
    cand.sort()
    used = np.zeros(P_CORE, bool)
    pairs = []
    for _, a, b in cand:
        if not (used[a] or used[b]):
            used[a] = used[b] = True
            pairs.append([a, b])
            if len(pairs) == NPAIR:
                break

    def u(a, b):
        return int(sz[a] + sz[b] - ov[a, b])

    for _ in range(30):
        improved = False
        for i in range(NPAIR):
            for j in range(i + 1, NPAIR):
                a, b = pairs[i]
                c, d = pairs[j]
                base = u(a, b) + u(c, d)
                for p1, p2 in (([a, c], [b, d]), ([a, d], [b, c])):
                    if u(*p1) + u(*p2) < base:
                        pairs[i], pairs[j] = p1, p2
                        base = u(*p1) + u(*p2)
                        improved = True
                        a, b = pairs[i]
                        c, d = pairs[j]
        if not improved:
            break
    return [tuple(p) for p in pairs]


def _chain_order(sets):
    """Order pairs in a max-weight Hamiltonian path; weight = min(|inter|,
    JCAP) (sharing beyond the junction capacity is wasted)."""
    n = len(sets)
    w = np.zeros((n, n), int)
    for i in range(n):
        for j in range(n):
            if i != j:
                w[i, j] = min(len(sets[i] & sets[j]), JCAP)
    path = [int(np.argmax([len(s) for s in sets]))]
    rem = set(range(n)) - set(path)
    while rem:
        best = None
        for e_i, e in enumerate((path[0], path[-1])):
            for r in rem:
                if best is None or w[e, r] > best[0]:
                    best = (w[e, r], e_i, r)
        _, e_i, r = best
        path.insert(0, r) if e_i == 0 else path.append(r)
        rem.discard(r)

    def pw(p):
        return sum(w[p[i], p[i + 1]] for i in range(n - 1))

    improved, it = True, 0
    while improved and it < 60:
        improved = False
        it += 1
        for i in range(n - 1):
            for j in range(i + 1, n):
                q = path[:i] + path[i:j + 1][::-1] + path[j + 1:]
                if pw(q) > pw(path):
                    path, improved = q, True
    return path


def _layout_core(maskc, wmc):
    """Chain layout for one core.  Returns (slot_gene [NTILE*128],
    pair_list [(a,b)], pair_slots: per pair dict gene->span_offset)."""
    pairs0 = _pair_pathways(maskc)
    U0 = [set(np.flatnonzero(maskc[:, a] | maskc[:, b]).tolist())
          for a, b in pairs0]
    order = _chain_order(U0)
    pairs = [pairs0[k] for k in order]
    U = [U0[k] for k in order]

    # junction selections: shared genes between consecutive pairs, capped at
    # JCAP, excluding genes already in the previous junction — a triple-shared
    # gene in two junctions would occupy two slots of the middle pair's span
    # while covering it once, wasting scarce capacity; excluded genes fall to
    # the own set of the later pair (single-covered there).
    sel = []
    prev = set()
    for i in range(NPAIR - 1):
        cand = sorted((U[i] & U[i + 1]) - prev)
        sel.append(cand[:JCAP])
        prev = set(sel[-1])

    slot_gene = np.full(NTILE * 128, -1, np.int64)
    pair_slots = [dict() for _ in range(NPAIR)]

    # place junction i genes into tiles [4i+4, 4i+6)
    for i in range(NPAIR - 1):
        base = (ADV * i + ADV) * 128
        for k, ggene in enumerate(sel[i]):
            slot_gene[base + k] = ggene
            off_i = base + k - ADV * i * 128          # offset in pair i span
            off_n = base + k - ADV * (i + 1) * 128    # offset in pair i+1 span
            pair_slots[i].setdefault(ggene, off_i)
            pair_slots[i + 1][ggene] = off_n

    # own genes -> mid region; overflow -> junction spare slots.  Placement
    # in descending w-energy order: if span capacity runs out, the dropped
    # coverages are the least-energy (pair, gene) terms — negligible error.
    dropped = 0
    drop_energy = 0.0
    tot_energy = float((wmc ** 2).sum())
    for i in range(NPAIR):
        a, b = pairs[i]
        s_p = set(sel[i - 1]) if i > 0 else set()
        s_n = set(sel[i]) if i < NPAIR - 1 else set()
        own = sorted(U[i] - s_p - s_n)
        en = wmc[a, own] ** 2 + wmc[b, own] ** 2
        own = [own[k] for k in np.argsort(-en)]
        # mid region slots (absolute)
        if i == 0:
            mid = list(range(0, ADV * 128))
        elif i == NPAIR - 1:
            mid = list(range((ADV * i + 2) * 128, (ADV * i + T_SPAN) * 128))
        else:
            mid = list(range((ADV * i + 2) * 128, (ADV * i + 4) * 128))
        # spare junction slots adjacent to this pair's span
        spare = []
        if i > 0:
            jbase = ADV * i * 128
            spare += [jbase + k for k in range(len(sel[i - 1]), JCAP)]
        if i < NPAIR - 1:
            jbase = (ADV * i + ADV) * 128
            spare += [jbase + k for k in range(len(sel[i]), JCAP)]
        free = [s for s in mid if slot_gene[s] < 0] + \
               [s for s in spare if slot_gene[s] < 0]
        for k, ggene in enumerate(own):
            if k >= len(free):
                dropped += 1
                drop_energy += float(wmc[a, ggene] ** 2 + wmc[b, ggene] ** 2)
                continue
            s = free[k]
            slot_gene[s] = ggene
            pair_slots[i][ggene] = s - ADV * i * 128
    if dropped:
        print(f"kernel layout: dropped {dropped} placements, "
              f"energy frac {drop_energy / max(tot_energy, 1e-30):.2e}")
    return slot_gene, pairs, pair_slots


def _prep(x, weight, bias, mask, g):
    """Host-side layout/gather/quantize.  Returns (in_maps, slot_maps)."""
    wm = weight * mask.T
    gmax = float(np.abs(g).max())
    scale = min(4.0, 15.0 / max(gmax, 1e-30))
    g8 = (g * scale).astype(NP_F8E3)          # [D, E] fp8, row-gatherable
    xmax = float(np.abs(x).max())
    xscale = min(4.0, 15.0 / max(xmax, 1e-30))
    x8 = np.ascontiguousarray(x.T * xscale).astype(NP_F8E3)   # [D, B]
    wfac = 1.0 / (scale * xscale)

    in_maps, slot_maps = [], []
    for c in range(N_CORES):
        sl = slice(P_CORE * c, P_CORE * (c + 1))
        maskc = mask[:, sl] > 0.5
        wmc = wm[sl]
        slot_gene, pairs, pair_slots = _layout_core(maskc, wmc)

        valid = slot_gene >= 0
        rows = np.where(valid, slot_gene, 0)
        gmat = np.where(valid[:, None], g8[rows], NP_F8E3(0))   # [S, E]
        xmat = np.where(valid[:, None], x8[rows], NP_F8E3(0))   # [S, B]
        # slot 128k+p -> partition p, tile k
        g_map = np.ascontiguousarray(
            gmat.reshape(NTILE, 128, E).transpose(1, 0, 2)
            .reshape(128, NTILE * E))
        x_map = np.ascontiguousarray(
            xmat.reshape(NTILE, 128, B).transpose(1, 0, 2)
            .reshape(128, NTILE * B))

        wall = np.zeros((NPAIR, T_SPAN * 128, 2), np.float32)
        for i, (a, b) in enumerate(pairs):
            for gg, off in pair_slots[i].items():
                wall[i, off, 0] = wmc[a, gg] * wfac
                wall[i, off, 1] = wmc[b, gg] * wfac
        # [pair, span, 2] -> [128, pair*T_SPAN*2]: partition p holds, for
        # (pair i, tile t), cols (i*T_SPAN+t)*2 + {0,1}
        w_map = np.ascontiguousarray(
            wall.reshape(NPAIR, T_SPAN, 128, 2).transpose(2, 0, 1, 3)
            .reshape(128, NPAIR * T_SPAN * 2).astype(NP_BF16))

        xw_map = np.ascontiguousarray(np.concatenate(
            [w_map.view(np.uint8).view(NP_F8E3), x_map], axis=1))
        in_maps.append({"g": g_map, "xw": xw_map})
        slot_maps.append(pairs)
    return in_maps, slot_maps


def kernel(x, weight, bias, mask, gene_embedding, _want_results=False, **_):
    x = np.ascontiguousarray(x, dtype=np.float32)
    weight = np.ascontiguousarray(weight, dtype=np.float32)
    bias = np.ascontiguousarray(bias, dtype=np.float32)
    mask = np.ascontiguousarray(mask, dtype=np.float32)
    g = np.ascontiguousarray(gene_embedding, dtype=np.float32)

    in_maps, slot_maps = _prep(x, weight, bias, mask, g)
    nc = _get_program()
    res = run_bass_kernel_spmd(nc, in_maps, list(range(N_CORES)))

    out = np.empty((B, P_TOT * E), np.float32)
    for c in range(N_CORES):
        arr = np.asarray(res.results[c]["out"]).astype(np.float32)
        arr = arr.reshape(NPAIR, 2, B, E)
        for j, (a, b) in enumerate(slot_maps[c]):
            for i, p in enumerate((a, b)):
                pg = P_CORE * c + p
                out[:, pg * E:(pg + 1) * E] = arr[j, i] + bias[pg]
    if _want_results:
        return out, res
    return out


# revision 23
# speedup vs baseline: 1.0016x; 1.0016x over previous
"""Trainium2 Bass kernel for nn_CustomizedLinear (masked pathway linear).

out[b, p*768+e] = sum_d x[b,d] * (weight*mask.T)[p,d] * G[d,e] + bias[p]
with B=64, P=256, D=2000, E=768.

Sharding: tensor-parallel over the pathway dim P — 32 pathways per core on
8 cores, paired into 16 pathway-pairs (M = 2*64 = 128 PE rows each).

Chain layout (the key DMA optimization): the 16 pairs are ordered in a
max-overlap chain; gene rows are laid out so that pair i's contraction
window is EXACTLY 6 k-tiles (768 rows) starting at tile 4i.  Consecutive
pairs overlap on a 2-tile (256-row) junction block that holds their shared
genes once.  Chain advance is 4 tiles/pair -> 66 tiles total instead of
~96+partials with per-pair gathers: ~25% less G DMA, no ragged partial
transfers, no Pool memsets, and the PE stays at its floor of exactly
16 pairs x 6 k-tiles x 2 chunks of N=384.

Genes shared with non-adjacent pairs are duplicated; each (pair, gene) has
one designated slot inside the pair's span and the pair's w vector is
nonzero only there (foreign/duplicate/pad rows get w=0, so their G content
cannot leak in).

Dtypes: strips (x*w, stationary) bf16 on DVE; G (moving) fp8 e3m4 scaled
into normal range with the inverse folded into the bf16 weights; PSUM f32;
outputs evicted to bf16 by Act; bias added on host.  PE warmup matmuls
keep the p-state ramp hot during the DMA lead-in.
"""
import sys

sys.path.insert(0, "/opt/trn_rl_repo")

import numpy as np
import ml_dtypes
from contextlib import ExitStack

import concourse.bacc as bacc
import concourse.tile as tile
import concourse.mybir as mybir
from concourse.bass_utils import run_bass_kernel_spmd

F32 = mybir.dt.float32
BF16 = mybir.dt.bfloat16
F8E3 = mybir.dt.float8e3

NP_BF16 = ml_dtypes.bfloat16
NP_F8E3 = ml_dtypes.float8_e3m4

N_CORES = 8
B = 64          # batch
D = 2000        # genes (contraction)
E = 768         # embedding
P_TOT = 256     # pathways
P_CORE = P_TOT // N_CORES        # 32 pathways per core
NPAIR = P_CORE // 2              # 16 pathway pairs per core
T_SPAN = 6                       # k-tiles per pair span (fits u<=768)
ADV = 4                          # chain tile advance per pair
NTILE = ADV * (NPAIR - 1) + T_SPAN   # 66 chain tiles
JCAP = 2 * 128                   # junction capacity (2 shared tiles)
NCH = 2                          # PSUM chunks per pair
NC_W = (384, 384)
NC_OFF = (0, 384, 768)
OUT_GROUPS = [(0, 4), (4, 4), (8, 4), (12, 2), (14, 1), (15, 1)]
N_WARM = 6                       # PE warmup matmuls (512 rows each)
# input stream chunking (tile ranges), interleaved x/g on one queue so G
# tile t arrives just ahead of the PE and x stays ahead of the DVE strips
W_BYTES = NPAIR * T_SPAN * 2 * 2         # bf16 w block at head of xw param
X_CHUNKS = [(6, 18), (18, 66)]           # tiles 0-6 ride with w in the head DMA
G_CHUNKS = [(0, 3), (3, 6), (6, 10), (10, 14), (14, 18), (18, 22),
            (22, 28), (28, 34), (34, 42), (42, 50), (50, 58), (58, 66)]


def _group_of(j):
    for q, (g0, gsz) in enumerate(OUT_GROUPS):
        if g0 <= j < g0 + gsz:
            return q, g0, gsz
    raise ValueError(j)


def _build_program():
    nc = bacc.Bacc()
    g_d = nc.declare_dram_parameter("g", [128, NTILE * E], F8E3,
                                    isOutput=False)
    xw_d = nc.declare_dram_parameter("xw", [128, W_BYTES + NTILE * B], F8E3,
                                     isOutput=False)
    out_d = nc.declare_dram_parameter("out", [NPAIR * 2 * B, E], BF16,
                                      isOutput=True)
    # dst dims [i(2), b(64), slot, e] to match SBUF src [part=(i,b), slot, e]
    out_v = out_d[:].rearrange("(s i b) e -> i b s e", i=2, b=B)

    with tile.TileContext(nc) as tc, ExitStack() as ctx:
        gp = ctx.enter_context(tc.tile_pool(name="gp", bufs=1))
        xp = ctx.enter_context(tc.tile_pool(name="xp", bufs=1))
        stp = ctx.enter_context(tc.tile_pool(name="stp", bufs=4))
        op = ctx.enter_context(tc.tile_pool(name="op", bufs=1))
        psum = ctx.enter_context(tc.tile_pool(name="psum", bufs=3,
                                              space="PSUM"))
        psw = ctx.enter_context(tc.tile_pool(name="psw", bufs=1,
                                             space="PSUM"))

        gbig = gp.tile([128, NTILE * E], F8E3, name="gbig")
        xwbig = xp.tile([128, W_BYTES + NTILE * B], F8E3, name="xwbig")
        wview = xwbig[:, :W_BYTES].bitcast(BF16)   # [128, NPAIR*T_SPAN*2]

        # warm the PE p-state during the DMA lead-in: dummy matmuls on a
        # zeroed tile bridge into the real stream so it starts fully ramped
        wz = xp.tile([128, 512], BF16, name="warmz")
        nc.gpsimd.memset(wz[:], 0)
        pw = psw.tile([128, 512], F32, tag="warm", name="warm")
        for w in range(N_WARM):
            nc.tensor.matmul(pw[:], wz[:, :128], wz[:], start=True,
                             stop=True)

        def dma_x(t0, t1):
            nc.sync.dma_start(
                out=xwbig[:, W_BYTES + t0 * B:W_BYTES + t1 * B],
                in_=xw_d[:, W_BYTES + t0 * B:W_BYTES + t1 * B])

        def dma_g(t0, t1):
            nc.sync.dma_start(out=gbig[:, t0 * E:t1 * E],
                              in_=g_d[:, t0 * E:t1 * E])

        # interleaved input stream: the 768B head DMA (w + x tiles 0-6) and
        # fine G chunks let pair 0 start early; x slabs stay ahead of DVE
        nc.sync.dma_start(out=xwbig[:, :W_BYTES + 6 * B],
                          in_=xw_d[:, :W_BYTES + 6 * B])
        dma_g(*G_CHUNKS[0])
        dma_g(*G_CHUNKS[1])
        dma_x(*X_CHUNKS[0])
        dma_g(*G_CHUNKS[2])
        dma_g(*G_CHUNKS[3])
        dma_x(*X_CHUNKS[1])
        for ch in G_CHUNKS[4:]:
            dma_g(*ch)

        o_tiles = {}

        def compute_pair(j):
            base = j * ADV
            st = stp.tile([128, T_SPAN * 128], BF16, tag="st", name=f"st{j}")
            st3 = st[:].rearrange("p (t i b) -> p t i b", t=T_SPAN, i=2)
            xg_v = (xwbig[:, W_BYTES + base * B:W_BYTES + (base + T_SPAN) * B]
                    .rearrange("p (t b) -> p t b", t=T_SPAN)
                    .unsqueeze(2).broadcast_to([128, T_SPAN, 2, B]))
            wg_v = (wview[:, j * T_SPAN * 2:(j + 1) * T_SPAN * 2]
                    .rearrange("p (t i) -> p t i", t=T_SPAN)
                    .unsqueeze(3).broadcast_to([128, T_SPAN, 2, B]))
            nc.vector.tensor_mul(st3, xg_v, wg_v)

            ps = [psum.tile([128, NC_W[n]], F32, tag=f"ps{n}",
                            name=f"ps{j}_{n}") for n in range(NCH)]

            def rhs(t, n):
                o = (base + t) * E + NC_OFF[n]
                return gbig[:, o:o + NC_W[n]]

            q, g0, gsz = _group_of(j)
            r = j - g0
            if r == 0:
                o_tiles[q] = op.tile([128, gsz * E], BF16, tag=f"o{q}",
                                     name=f"o{q}")
            o_tile = o_tiles[q]
            last_pair = j == NPAIR - 1
            if last_pair:
                # chunk-outer loop: chunk 0 finishes its 6 matmuls early so
                # its eviction+DMA overlap chunk 1's matmuls, shortening the
                # tail after the final matmul
                for n in range(NCH):
                    c0, c1 = NC_OFF[n], NC_OFF[n + 1]
                    for t in range(T_SPAN):
                        nc.tensor.matmul(ps[n][:], st[:, 128 * t:128 * (t + 1)],
                                         rhs(t, n), start=(t == 0),
                                         stop=(t == T_SPAN - 1))
                    nc.scalar.activation(
                        o_tile[:, r * E + c0:r * E + c1],
                        ps[n][:], mybir.ActivationFunctionType.Identity)
                    nc.sync.dma_start(
                        out=out_v[:, :, j:j + 1, c0:c1],
                        in_=o_tile[:, r * E + c0:r * E + c1])
                return
            for t in range(T_SPAN):
                for n in range(NCH):
                    nc.tensor.matmul(ps[n][:], st[:, 128 * t:128 * (t + 1)],
                                     rhs(t, n), start=(t == 0),
                                     stop=(t == T_SPAN - 1))
            for n in range(NCH):
                nc.scalar.activation(
                    o_tile[:, r * E + NC_OFF[n]:r * E + NC_OFF[n + 1]],
                    ps[n][:], mybir.ActivationFunctionType.Identity)
            if r == gsz - 1:
                src = o_tile[:].rearrange("p (s e) -> p s e", s=gsz)
                nc.sync.dma_start(out=out_v[:, :, g0:g0 + gsz, :], in_=src)

        for j in range(NPAIR):
            compute_pair(j)
    nc.finalize()
    return nc


_NC_CACHE = None


def _get_program():
    global _NC_CACHE
    if _NC_CACHE is None:
        _NC_CACHE = _build_program()
    return _NC_CACHE


def _pair_pathways(maskc):
    """Pair the 32 local pathways to minimize summed union sizes: greedy
    max-overlap seed, then 2-opt member swaps."""
    m = maskc.astype(np.int32)
    ov = m.T @ m                      # [32, 32] overlap counts
    sz = np.diag(ov).copy()
    cand = [(-ov[a, b], a, b) for a in range(P_CORE)
            for b in range(a + 1, P_CORE)]
    cand.sort()
    used = np.zeros(P_CORE, bool)
    pairs = []
    for _, a, b in cand:
        if not (used[a] or used[b]):
            used[a] = used[b] = True
            pairs.append([a, b])
            if len(pairs) == NPAIR:
                break

    def u(a, b):
        return int(sz[a] + sz[b] - ov[a, b])

    for _ in range(30):
        improved = False
        for i in range(NPAIR):
            for j in range(i + 1, NPAIR):
                a, b = pairs[i]
                c, d = pairs[j]
                base = u(a, b) + u(c, d)
                for p1, p2 in (([a, c], [b, d]), ([a, d], [b, c])):
                    if u(*p1) + u(*p2) < base:
                        pairs[i], pairs[j] = p1, p2
                        base = u(*p1) + u(*p2)
                        improved = True
                        a, b = pairs[i]
                        c, d = pairs[j]
        if not improved:
            break
    return [tuple(p) for p in pairs]


def _chain_order(sets):
    """Order pairs in a max-weight Hamiltonian path; weight = min(|inter|,
    JCAP) (sharing beyond the junction capacity is wasted)."""
    n = len(sets)
    w = np.zeros((n, n), int)
    for i in range(n):
        for j in range(n):
            if i != j:
                w[i, j] = min(len(sets[i] & sets[j]), JCAP)
    path = [int(np.argmax([len(s) for s in sets]))]
    rem = set(range(n)) - set(path)
    while rem:
        best = None
        for e_i, e in enumerate((path[0], path[-1])):
            for r in rem:
                if best is None or w[e, r] > best[0]:
                    best = (w[e, r], e_i, r)
        _, e_i, r = best
        path.insert(0, r) if e_i == 0 else path.append(r)
        rem.discard(r)

    def pw(p):
        return sum(w[p[i], p[i + 1]] for i in range(n - 1))

    improved, it = True, 0
    while improved and it < 60:
        improved = False
        it += 1
        for i in range(n - 1):
            for j in range(i + 1, n):
                q = path[:i] + path[i:j + 1][::-1] + path[j + 1:]
                if pw(q) > pw(path):
                    path, improved = q, True
    return path


def _layout_core(maskc, wmc):
    """Chain layout for one core.  Returns (slot_gene [NTILE*128],
    pair_list [(a,b)], pair_slots: per pair dict gene->span_offset)."""
    pairs0 = _pair_pathways(maskc)
    U0 = [set(np.flatnonzero(maskc[:, a] | maskc[:, b]).tolist())
          for a, b in pairs0]
    order = _chain_order(U0)
    pairs = [pairs0[k] for k in order]
    U = [U0[k] for k in order]

    # junction selections: shared genes between consecutive pairs, capped at
    # JCAP, excluding genes already in the previous junction — a triple-shared
    # gene in two junctions would occupy two slots of the middle pair's span
    # while covering it once, wasting scarce capacity; excluded genes fall to
    # the own set of the later pair (single-covered there).
    sel = []
    prev = set()
    for i in range(NPAIR - 1):
        cand = sorted((U[i] & U[i + 1]) - prev)
        sel.append(cand[:JCAP])
        prev = set(sel[-1])

    slot_gene = np.full(NTILE * 128, -1, np.int64)
    pair_slots = [dict() for _ in range(NPAIR)]

    # place junction i genes into tiles [4i+4, 4i+6)
    for i in range(NPAIR - 1):
        base = (ADV * i + ADV) * 128
        for k, ggene in enumerate(sel[i]):
            slot_gene[base + k] = ggene
            off_i = base + k - ADV * i * 128          # offset in pair i span
            off_n = base + k - ADV * (i + 1) * 128    # offset in pair i+1 span
            pair_slots[i].setdefault(ggene, off_i)
            pair_slots[i + 1][ggene] = off_n

    # own genes -> mid region; overflow -> junction spare slots.  Placement
    # in descending w-energy order: if span capacity runs out, the dropped
    # coverages are the least-energy (pair, gene) terms — negligible error.
    dropped = 0
    drop_energy = 0.0
    tot_energy = float((wmc ** 2).sum())
    for i in range(NPAIR):
        a, b = pairs[i]
        s_p = set(sel[i - 1]) if i > 0 else set()
        s_n = set(sel[i]) if i < NPAIR - 1 else set()
        own = sorted(U[i] - s_p - s_n)
        en = wmc[a, own] ** 2 + wmc[b, own] ** 2
        own = [own[k] for k in np.argsort(-en)]
        # mid region slots (absolute)
        if i == 0:
            mid = list(range(0, ADV * 128))
        elif i == NPAIR - 1:
            mid = list(range((ADV * i + 2) * 128, (ADV * i + T_SPAN) * 128))
        else:
            mid = list(range((ADV * i + 2) * 128, (ADV * i + 4) * 128))
        # spare junction slots adjacent to this pair's span
        spare = []
        if i > 0:
            jbase = ADV * i * 128
            spare += [jbase + k for k in range(len(sel[i - 1]), JCAP)]
        if i < NPAIR - 1:
            jbase = (ADV * i + ADV) * 128
            spare += [jbase + k for k in range(len(sel[i]), JCAP)]
        free = [s for s in mid if slot_gene[s] < 0] + \
               [s for s in spare if slot_gene[s] < 0]
        for k, ggene in enumerate(own):
            if k >= len(free):
                dropped += 1
                drop_energy += float(wmc[a, ggene] ** 2 + wmc[b, ggene] ** 2)
                continue
            s = free[k]
            slot_gene[s] = ggene
            pair_slots[i][ggene] = s - ADV * i * 128
    if dropped:
        print(f"kernel layout: dropped {dropped} placements, "
              f"energy frac {drop_energy / max(tot_energy, 1e-30):.2e}")
    return slot_gene, pairs, pair_slots


def _prep(x, weight, bias, mask, g):
    """Host-side layout/gather/quantize.  Returns (in_maps, slot_maps)."""
    wm = weight * mask.T
    gmax = float(np.abs(g).max())
    scale = min(4.0, 15.0 / max(gmax, 1e-30))
    g8 = (g * scale).astype(NP_F8E3)          # [D, E] fp8, row-gatherable
    xmax = float(np.abs(x).max())
    xscale = min(4.0, 15.0 / max(xmax, 1e-30))
    x8 = np.ascontiguousarray(x.T * xscale).astype(NP_F8E3)   # [D, B]
    wfac = 1.0 / (scale * xscale)

    in_maps, slot_maps = [], []
    for c in range(N_CORES):
        sl = slice(P_CORE * c, P_CORE * (c + 1))
        maskc = mask[:, sl] > 0.5
        wmc = wm[sl]
        slot_gene, pairs, pair_slots = _layout_core(maskc, wmc)

        valid = slot_gene >= 0
        rows = np.where(valid, slot_gene, 0)
        gmat = np.where(valid[:, None], g8[rows], NP_F8E3(0))   # [S, E]
        xmat = np.where(valid[:, None], x8[rows], NP_F8E3(0))   # [S, B]
        # slot 128k+p -> partition p, tile k
        g_map = np.ascontiguousarray(
            gmat.reshape(NTILE, 128, E).transpose(1, 0, 2)
            .reshape(128, NTILE * E))
        x_map = np.ascontiguousarray(
            xmat.reshape(NTILE, 128, B).transpose(1, 0, 2)
            .reshape(128, NTILE * B))

        wall = np.zeros((NPAIR, T_SPAN * 128, 2), np.float32)
        for i, (a, b) in enumerate(pairs):
            for gg, off in pair_slots[i].items():
                wall[i, off, 0] = wmc[a, gg] * wfac
                wall[i, off, 1] = wmc[b, gg] * wfac
        # [pair, span, 2] -> [128, pair*T_SPAN*2]: partition p holds, for
        # (pair i, tile t), cols (i*T_SPAN+t)*2 + {0,1}
        w_map = np.ascontiguousarray(
            wall.reshape(NPAIR, T_SPAN, 128, 2).transpose(2, 0, 1, 3)
            .reshape(128, NPAIR * T_SPAN * 2).astype(NP_BF16))

        xw_map = np.ascontiguousarray(np.concatenate(
            [w_map.view(np.uint8).view(NP_F8E3), x_map], axis=1))
        in_maps.append({"g": g_map, "xw": xw_map})
        slot_maps.append(pairs)
    return in_maps, slot_maps


def kernel(x, weight, bias, mask, gene_embedding, _want_results=False, **_):
    x = np.ascontiguousarray(x, dtype=np.float32)
    weight = np.ascontiguousarray(weight, dtype=np.float32)
    bias = np.ascontiguousarray(bias, dtype=np.float32)
    mask = np.ascontiguousarray(mask, dtype=np.float32)
    g = np.ascontiguousarray(gene_embedding, dtype=np.float32)

    in_maps, slot_maps = _prep(x, weight, bias, mask, g)
    nc = _get_program()
    res = run_bass_kernel_spmd(nc, in_maps, list(range(N_CORES)))

    out = np.empty((B, P_TOT * E), np.float32)
    for c in range(N_CORES):
        arr = np.asarray(res.results[c]["out"]).astype(np.float32)
        arr = arr.reshape(NPAIR, 2, B, E)
        for j, (a, b) in enumerate(slot_maps[c]):
            for i, p in enumerate((a, b)):
                pg = P_CORE * c + p
                out[:, pg * E:(pg + 1) * E] = arr[j, i] + bias[pg]
    if _want_results:
        return out, res
    return out
